# revision 46
# baseline (speedup 1.0000x reference)
"""MoE layer (top-k routing) on 8 Trainium2 NeuronCores.

Expert-parallel per the sharding hint: the host computes router softmax +
top-k (0.1% of FLOPs) and realizes the "all-to-all dispatch by expert
assignment" while building the per-core SPMD input maps; each core runs
expert FFN work in bf16 (fp32 PSUM accumulation); the host applies the
combine weights and scatter-adds results back to [B,N,C].

Load balance: each expert's FFN is split along D_FF into four quarter-units
(exact: gelu is elementwise over F and GEMM2 contracts F, so the four
partial y's just add). The 32 quarter-units are assigned four per core, one
per slot class A-D: slot A holds the two largest experts' quarters, slot B
the next two, etc. Each slot is padded to the max count within its pair, so
per-core padded work is sum over slots of max(pair) — within ~1% of the
perfect-balance floor — instead of 2*max(all counts).

Perf notes (measured on HW, ~922.5us vs ~874us pure-matmul floor):
- PE is the bottleneck: 95%+ matmul occupancy, 215ns per 128x128x512
  bf16 matmul (512 cycles @2.4GHz + 2.5ns NX dispatch floor). fp8 would
  halve compute but fails the 2e-2 rel-err gate (measured 4.6e-2);
  1024-wide moving ops fail walrus ISA codegen (s3d3_mm_num_elements).
- The startup window (~9-30us) is DMA-limited while the DMA subsystem
  ramps (~125GB/s aggregate early): tile-0's x0 + w1A + w2A (5MB) are
  chunked per contract-block across all three issuing queues (sync /
  scalar / gpsimd) in consumption order. The scalar queue carries only
  4 early dma_starts - each one blocks the ACT engine ~2us and would
  starve gelu otherwise.
- All DRAM tensors are host-repacked: x/y chunk-major [128,nt,8,512]
  (8KB descriptors/partition-line per tile), weights [128,8,1024] (16KB),
  biases pre-transposed into one [128,64] blob (one 128-descriptor DMA
  vs 8x1024 4-byte scatters). DMA engines are descriptor-rate-bound, so
  fat descriptors matter more than queue choice.
- 20 zero-input warm-up matmuls run during the load window: a cold/idled
  PE runs 2-3x slow (427-609ns) for its first ~15 matmuls.
- x tiles 0-2 prefetch ahead of the 12MB B-D weight bulk (an 8.8us PE
  stall when queued behind it); the bulk is split sync/gpsimd.
- Remaining gap to floor: ~9us fixed NEFF preamble + ring start, ~7us
  of early DMA-ramp stalls, ~10.9us NX dispatch tax, ~7us pair-padding
  (SPMD uniform caps), ~5us drain. All four variants of early-window
  scheduling measured within +/-1.5us - it is delivery-bound, not
  schedule-bound.
"""

import json
import os
import sys
import types

import numpy as np
import ml_dtypes

D_MODEL = 1024
D_FF = 4096
N_EXPERTS = 8
N_CORES = 8

P = 128
CB = D_MODEL // P      # 8 c-blocks of 128
FQ = D_FF // 4         # F quarter = 1024
FBQ = FQ // P          # 8 f-blocks per quarter
TN = 512               # token tile (matmul moving free dim / one PSUM bank)
SLOTS = ("A", "B", "C", "D", "E")   # E = overflow unit (tiny)


def _shim_axon_hooks():
    """Register the NTFF profile hook bass_utils looks for under axon; the
    image's `antenv` stub lacks `axon_hooks`."""
    if "antenv.axon_hooks" in sys.modules:
        return
    try:
        import trn_agent_boot.trn_boot as _tb
        hook = _tb._ntff_profile_via_ctypes("/opt/axon/libaxon_pjrt.so")
    except Exception:
        hook = None
    mod = types.ModuleType("antenv.axon_hooks")
    mod.get_axon_ntff_profile_hook = lambda: hook
    mod.set_axon_ntff_profile_hook = lambda h: None
    sys.modules["antenv.axon_hooks"] = mod


_shim_axon_hooks()

import concourse.bass as bass            # noqa: E402
import concourse.tile as tile            # noqa: E402
from concourse import mybir              # noqa: E402
from concourse.bass import ds, ts        # noqa: E402
from concourse.bass_utils import run_bass_kernel_spmd  # noqa: E402


def _fix_multiwait_bir(nc):
    """Split instructions carrying >1 sync wait (the TileContext tail drain)
    into single-wait NoOps; this walrus build rejects multi-wait CTRL
    instructions."""
    raw = bass.Bass.to_json_bytes(nc)
    d = json.loads(raw)
    for f in d["functions"]:
        for b in f["blocks"]:
            out = []
            for i in b["instructions"]:
                si = i.get("sync_info") or {}
                waits = si.get("on_wait") or []
                if len(waits) > 1:
                    for k, w in enumerate(waits[:-1]):
                        out.append({
                            "name": f"{i['name']}_wsplit{k}",
                            "engine": i["engine"],
                            "ins": [], "outs": [],
                            "opcode": "NoOp",
                            "sync_info": {"on_update": [], "on_wait": [w]},
                        })
                    si["on_wait"] = [waits[-1]]
                out.append(i)
            b["instructions"] = out
    fixed = json.dumps(d).encode()
    nc.to_json_bytes = lambda: fixed


_NC_CACHE = {}


def _token_tiles(cap, small_first=False):
    # small_first: a 256-token leading tile halves the bytes the very first
    # matmul waits on; later tiles are prefetched early enough to cover it
    tiles, off = [], 0
    if small_first and cap > TN:
        tiles.append((0, 256))
        off = 256
    while off < cap:
        tw = min(TN, cap - off)
        tiles.append((off, tw))
        off += tw
    return tiles


def _build_moe_kernel(caps):
    """Four quarter-expert FFN units per core (slots A-D), SPMD x8."""
    key = tuple(caps)
    if key in _NC_CACHE:
        return _NC_CACHE[key]

    bf16 = mybir.dt.bfloat16
    f32 = mybir.dt.float32
    Act = mybir.ActivationFunctionType

    nc = bass.Bass("TRN2", target_bir_lowering=False, debug=False,
                   num_devices=N_CORES)

    # all 8 per-slot bias vectors pre-rearranged by the host into one
    # [128, 64] blob: one 128-descriptor DMA instead of 8 DMAs of 1024
    # 4-byte descriptors each (~0.8us queue time + heavy DMA-engine load)
    biasP = nc.declare_dram_parameter("bias", [P, len(SLOTS) * (FBQ + CB)],
                                      f32, isOutput=False)
    # All DRAM tensors are host-repacked to partition-major layouts so each
    # DMA descriptor covers a full per-partition line (8-16KB) instead of a
    # 1-2KB row segment: the DMA system is descriptor-rate-bound (~80ns per
    # descriptor per engine), not byte-bound, so fat descriptors are what
    # make loads land fast.
    units = []
    for si, (slot, cap) in enumerate(zip(SLOTS, caps)):
        u = {"cap": cap, "slot": slot, "boff": si * (FBQ + CB)}
        u["tiles"] = _token_tiles(cap)
        nt = len(u["tiles"])
        u["nt"] = nt
        # x/y chunk-major: [128, ntiles, 8, 512] -> per-(p,tile) 8KB runs
        u["xT"] = nc.declare_dram_parameter(f"xT{slot}", [P, nt, CB, TN], bf16, isOutput=False)
        # w: [128, 8, 1024] -> per-p 16KB runs
        u["w1t"] = nc.declare_dram_parameter(f"w1t{slot}", [P, CB, FQ], bf16, isOutput=False)
        u["w2t"] = nc.declare_dram_parameter(f"w2t{slot}", [P, FBQ, D_MODEL], bf16, isOutput=False)
        # partials return as bf16: halves the output DMA so total traffic
        # stays under the chip's P0 power-throttle trigger (observed: the
        # f32 version pushed PE from 2.4 to 2.0 GHz); host sums in f32
        u["yT"] = nc.declare_dram_parameter(f"yT{slot}", [P, nt, CB, TN], bf16, isOutput=True)
        u["xr"] = u["xT"].ap()      # [128, nt, 8, 512]
        u["w1r"] = u["w1t"].ap()    # [128, 8, 1024]
        u["w2r"] = u["w2t"].ap()    # [128, 8, 1024]
        u["yr"] = u["yT"].ap()
        units.append(u)

    with tile.TileContext(nc) as tc:
        with (
            tc.tile_pool(name="weights", bufs=1) as wpool,
            tc.tile_pool(name="xin", bufs=3) as xpool,
            tc.tile_pool(name="hbuf", bufs=1) as hpool,
            tc.tile_pool(name="yout", bufs=3) as ypool,
            tc.tile_pool(name="psum", bufs=4, space="PSUM") as psum,
            tc.tile_pool(name="warm", bufs=1) as warmpool,
        ):
            # PE clock warm-up: ~20 dependency-free matmuls on a memset tile
            # run while the first loads are in flight. A cold PE runs its
            # first ~15 matmuls at half clock (427-609ns vs 215ns); burning
            # the ramp on dummies makes the real tile-0 matmuls full-speed.
            wsrc = warmpool.tile([P, P + TN], bf16, tag="wsrc", name="wsrc")
            nc.vector.memset(wsrc[:, :], 0)
            # borrow a rotation slot from the main psum pool's ph tag; the
            # warm matmuls retire long before the 4-deep rotation reuses it
            wps = psum.tile([P, TN], f32, tag="ph", name="wps")
            for _ in range(20):
                nc.tensor.matmul(wps[:, :], lhsT=wsrc[:, 0:P],
                                 rhs=wsrc[:, P:P + TN], start=True, stop=True)
            # ---- loads. SP-ring FIFO order is chosen so PE never waits:
            # tile-0 tokens + first w1 strip first (w1 strip on the ACT ring
            # so it overlaps x0's load), then unit A's remaining weights,
            # then unit A's tile-1 tokens BEFORE units B-D's weight bulk so
            # early tiles stay ahead of the PE.
            # chunked so the first matmul waits on 128KB, not a 1MB monolith:
            # x0 per-k on the sync ring; w1A/w2A chunked along the contract
            # group g (keeps 2KB descriptors — chunking along f/c would 8x
            # the descriptor count). w1A is split across the gpsimd and
            # scalar rings so G1(tile0) is fed at ~2 rings' bandwidth. The
            # scalar queue gets only 4 early dma_starts (issued well before
            # the first gelu needs the engine).
            ua = units[0]
            ua["x0"] = xpool.tile([P, CB, TN], bf16, tag="xt", name="x0A")
            ua["w1_sb"] = wpool.tile([P, CB, FQ], bf16, tag="w1A", name="w1A")
            # early window is aggregate-DMA-ramp-limited (~125GB/s): feed
            # tile-0's consumption order with many small per-g chunks spread
            # across all three rings (empirically the fastest variant): x0
            # per-g on sync, w1A per-g alternating gpsimd/scalar, w2A per-g
            # on gpsimd behind the bias blob.
            bias_sb = wpool.tile([P, len(SLOTS) * (FBQ + CB)], f32, tag="bias",
                                 name="bias")
            nc.gpsimd.dma_start(bias_sb[:, :], biasP.ap())
            for g in range(CB):
                # first-needed chunks on scalar: its ring starts ~2us
                # earlier than gpsimd's
                ring = nc.scalar if g < 4 else nc.gpsimd
                ring.dma_start(ua["w1_sb"][:, g, :], ua["w1r"][:, g, :])
            for g in range(CB):
                nc.sync.dma_start(ua["x0"][:, g, :], ua["xr"][:, 0, g, :])
            ua["w2_sb"] = wpool.tile([P, FBQ, D_MODEL], bf16, tag="w2A", name="w2A")
            for g in range(FBQ):
                nc.gpsimd.dma_start(ua["w2_sb"][:, g, :], ua["w2r"][:, g, :])

            # unit A tiles 1 and 2 ahead of the remaining weight bulk (the
            # tile-2 load sat behind 12MB of B-D weights on the sync ring and
            # cost an 8.8us PE stall)
            for ti in (1, 2):
                if len(ua["tiles"]) > ti:
                    xt = xpool.tile([P, CB, TN], bf16, tag="xt", name=f"x{ti}A")
                    nc.sync.dma_start(xt[:, :, :], ua["xr"][:, ti, :, :])
                    ua[f"x{ti}"] = xt

            # B/C weight bulk split across the sync and gpsimd rings. Units
            # D and E REUSE earlier units' weight buffers via tag sharing
            # (saves 64KB/partition of SBUF): their loads sit on the
            # otherwise-idle gpsimd queue and block there until the donor's
            # last read — donors free at ~217-690us, consumers start at
            # ~690-910us, so the serialization is harmless.
            for u in units[1:3]:
                slot = u["slot"]
                u["w1_sb"] = wpool.tile([P, CB, FQ], bf16, tag=f"w1{slot}",
                                        name=f"w1{slot}")
                u["w2_sb"] = wpool.tile([P, FBQ, D_MODEL], bf16, tag=f"w2{slot}",
                                        name=f"w2{slot}")
            ud, ue = units[3], units[4]
            for u in units[1:3]:
                nc.sync.dma_start(u["w1_sb"][:, :, :], u["w1r"][:, :, :])
                nc.gpsimd.dma_start(u["w2_sb"][:, :, :], u["w2r"][:, :, :])

            # ---- compute: unit A..E tiles in sequence. Deferred weight
            # loads for D and E reuse donor buffers (tag sharing) and MUST
            # be emitted after the donor unit's matmuls so the WAR hazard
            # is tracked in program order; they sit on the idle gpsimd
            # queue and wait there for the donor's last read.
            for ui, u in enumerate(units):
                if ui == 3:
                    # donors: w1A free after A's G1, w2B free after B's G2
                    ud["w1_sb"] = wpool.tile([P, CB, FQ], bf16, tag="w1A",
                                             name="w1D")
                    nc.gpsimd.dma_start(ud["w1_sb"][:, :, :], ud["w1r"][:, :, :])
                    ud["w2_sb"] = wpool.tile([P, FBQ, D_MODEL], bf16,
                                             tag="w2B", name="w2D")
                    nc.gpsimd.dma_start(ud["w2_sb"][:, :, :], ud["w2r"][:, :, :])
                if ui == 4:
                    ue["w1_sb"] = wpool.tile([P, CB, FQ], bf16, tag="w1B",
                                             name="w1E")
                    nc.gpsimd.dma_start(ue["w1_sb"][:, :, :], ue["w1r"][:, :, :])
                    ue["w2_sb"] = wpool.tile([P, FBQ, D_MODEL], bf16,
                                             tag="w2C", name="w2E")
                    nc.gpsimd.dma_start(ue["w2_sb"][:, :, :], ue["w2r"][:, :, :])
                for ti, (off, tw) in enumerate(u["tiles"]):
                    if f"x{ti}" in u:
                        xt = u[f"x{ti}"]
                    else:
                        xt = xpool.tile([P, CB, TN], bf16, tag="xt")
                        nc.sync.dma_start(xt[:, :, :], u["xr"][:, ti, :, :])

                    ht = hpool.tile([P, FBQ, TN], bf16, tag="ht")
                    for m in range(FBQ):
                        ph = psum.tile([P, TN], f32, tag="ph")
                        for k in range(CB):
                            nc.tensor.matmul(
                                ph[:, :tw],
                                lhsT=u["w1_sb"][:, k, ts(m, P)],
                                rhs=xt[:, k, :tw],
                                start=(k == 0), stop=(k == CB - 1),
                            )
                        nc.scalar.activation(ht[:, m, :tw], ph[:, :tw], Act.Gelu,
                                             bias=bias_sb[:, u["boff"] + m:u["boff"] + m + 1])

                    last = (u is units[-1]) and (ti == len(u["tiles"]) - 1)
                    yt = ypool.tile([P, CB, TN], bf16, tag="yt")
                    for c in range(CB):
                        py = psum.tile([P, TN], f32, tag="py")
                        for k in range(FBQ):
                            nc.tensor.matmul(
                                py[:, :tw],
                                lhsT=u["w2_sb"][:, k, ts(c, P)],
                                rhs=ht[:, k, :tw],
                                start=(k == 0), stop=(k == FBQ - 1),
                            )
                        bo = u["boff"] + FBQ
                        nc.scalar.add(yt[:, c, :tw], py[:, :tw],
                                      bias_sb[:, bo + c:bo + c + 1])
                        if last and tw >= 256:
                            # final tile: per-block stores overlap the tail
                            # GEMM2 blocks instead of one post-loop DMA
                            nc.sync.dma_start(u["yr"][:, ti, c, :tw],
                                              yt[:, c, :tw])
                    if not last or tw < 256:
                        # full-chunk store (8KB descriptors); the pad zone
                        # carries stale data the host never reads. For a
                        # small final tile one store beats 8 serial 0.6us
                        # DIRECT2D issues after the last matmul.
                        nc.sync.dma_start(u["yr"][:, ti, :, :], yt[:, :, :])

    _fix_multiwait_bir(nc)
    _NC_CACHE[key] = nc
    return nc


def _route(xf, router_w, k):
    """Replicate the reference router numerics (f32 softmax, top-k, renorm)."""
    logits = xf @ router_w.T.astype(np.float32)          # [T, E]
    m = logits.max(axis=-1, keepdims=True)
    e = np.exp(logits - m, dtype=np.float32)
    probs = e / e.sum(axis=-1, keepdims=True)
    # descending, ties -> lower index first (matches jax.lax.top_k)
    idx = np.argsort(-probs, axis=-1, kind="stable")[:, :k]   # [T, k]
    w = np.take_along_axis(probs, idx, axis=-1)               # [T, k]
    w = w / (w.sum(axis=-1, keepdims=True) + 1e-9)
    return idx, w


def _align16(n):
    # 4-token (8-byte) alignment keeps DMA rows aligned; finer than 16
    # saves ~20 padded tokens across the four slots
    return max(P, -(-n // 4) * 4)


def kernel(x, router_w, expert_w1, expert_b1, expert_w2, expert_b2, top_k):
    x = np.asarray(x)
    router_w = np.asarray(router_w, dtype=np.float32)
    expert_w1 = np.asarray(expert_w1, dtype=np.float32)
    expert_b1 = np.asarray(expert_b1, dtype=np.float32)
    expert_w2 = np.asarray(expert_w2, dtype=np.float32)
    expert_b2 = np.asarray(expert_b2, dtype=np.float32)
    k = int(np.asarray(top_k))
    Bq, Nq, C = x.shape
    Tq = Bq * Nq
    E = expert_w1.shape[0]
    xf = np.ascontiguousarray(x.reshape(Tq, C), dtype=np.float32)

    idx, w = _route(xf, router_w, k)

    tok_idx, tok_w = [], []
    for e in range(E):
        mask = idx == e
        sel = np.nonzero(mask.any(axis=-1))[0]
        tok_idx.append(sel)
        tok_w.append((w * mask).sum(axis=-1)[sel].astype(np.float32))
    counts = np.array([len(s) for s in tok_idx])

    # Slots A-D pair experts by rank but pad only to the SMALLER member of
    # the A and D pairs; the two truncated experts' overflow tokens (rank 0
    # on cores 0-3, rank 6 on cores 4-7 — slot D is swapped so rank 6 sits
    # on the 4-7 half) run as a tiny fifth unit E with its own weights.
    order = np.argsort(-counts, kind="stable")
    ranks = [int(order[i]) for i in range(8)]
    caps = [_align16(int(counts[ranks[1]])),
            _align16(int(counts[ranks[2]])),
            _align16(int(counts[ranks[4]])),
            _align16(int(counts[ranks[7]]))]
    ov_a = int(counts[ranks[0]]) - caps[0]
    ov_d = int(counts[ranks[6]]) - caps[3]
    caps.append(max(4, -(-max(ov_a, ov_d, 1) // 4) * 4))

    nc = _build_moe_kernel(tuple(caps))

    def _pack(tmp, nt):
        return np.ascontiguousarray(
            tmp.reshape(nt, TN, CB, P).transpose(3, 0, 2, 1)
        ).astype(ml_dtypes.bfloat16)

    # slot -> (expert on cores 0-3, expert on cores 4-7)
    slot_experts = [(ranks[0], ranks[1]), (ranks[2], ranks[3]),
                    (ranks[4], ranks[5]), (ranks[7], ranks[6])]

    # one xT per expert (first main_cap tokens), shared by its 4 quarters
    xTs, main_of = {}, {}
    for s in range(4):
        nt = (caps[s] + TN - 1) // TN
        for e in slot_experts[s]:
            m = min(int(counts[e]), caps[s])
            main_of[e] = (s, m)
            tmp = np.zeros((nt * TN, C), dtype=np.float32)
            tmp[:m] = xf[tok_idx[e][:m]]
            xTs[e] = _pack(tmp, nt)
    # overflow xT per core-half
    nt_e = (caps[4] + TN - 1) // TN
    xTov = []
    for e in (ranks[0], ranks[6]):
        s, m = main_of[e]
        k = int(counts[e]) - m
        tmp = np.zeros((nt_e * TN, C), dtype=np.float32)
        if k > 0:
            tmp[:k] = xf[tok_idx[e][m:]]
        xTov.append(_pack(tmp, nt_e))

    in_maps = [dict() for _ in range(N_CORES)]
    placement = {}          # (expert, quarter) -> (core, slot name)
    FBQ_, CB_ = FQ // P, C // P
    blobs = [np.zeros((P, len(SLOTS) * (FBQ_ + CB_)), dtype=np.float32)
             for _ in range(N_CORES)]
    for s, slot in enumerate(SLOTS):
        for core in range(N_CORES):
            q = core % 4
            half = 0 if core < 4 else 1
            if s < 4:
                e = slot_experts[s][half]
                placement[(e, q)] = (core, slot)
                xarr = xTs[e]
            else:
                e = ranks[0] if half == 0 else ranks[6]
                xarr = xTov[half]
            f0, f1 = q * FQ, (q + 1) * FQ
            b2 = expert_b2[e] if q == 0 else np.zeros(C, dtype=np.float32)
            bo = s * (FBQ_ + CB_)
            blobs[core][:, bo:bo + FBQ_] = expert_b1[e, f0:f1].reshape(FBQ_, P).T
            blobs[core][:, bo + FBQ_:bo + FBQ_ + CB_] = b2.reshape(CB_, P).T
            w1q = expert_w1[e, f0:f1].T.reshape(CB_, P, FQ).transpose(1, 0, 2)
            w2q = expert_w2[e, :, f0:f1].T.reshape(FBQ_, P, C).transpose(1, 0, 2)
            in_maps[core].update({
                f"xT{slot}": xarr,
                f"w1t{slot}": np.ascontiguousarray(w1q).astype(ml_dtypes.bfloat16),
                f"w2t{slot}": np.ascontiguousarray(w2q).astype(ml_dtypes.bfloat16),
            })
    for core in range(N_CORES):
        in_maps[core]["bias"] = blobs[core]

    trace = os.environ.get("BASS_MOE_TRACE") == "1"
    res = run_bass_kernel_spmd(
        nc, in_maps, core_ids=list(range(N_CORES)),
        trace=trace,
        tmpdir=os.environ.get("BASS_MOE_TMPDIR") if trace else None,
    )
    if trace:
        kernel.last_exec_time_ns = res.exec_time_ns
        kernel.last_trace = (res.instructions_and_trace or (None, None))[1]

    def _unpack(core, slot, take):
        yq = res.results[core][f"yT{slot}"]            # [128, nt, 8, 512]
        nt = yq.shape[1]
        return yq.transpose(1, 3, 2, 0).reshape(nt * TN, C)[:take].astype(np.float32)

    out = np.zeros((Tq, C), dtype=np.float32)
    for e in range(E):
        cnt = int(counts[e])
        if not cnt:
            continue
        _, m = main_of[e]
        acc = np.zeros((m, C), dtype=np.float32)
        for q in range(4):
            core, slot = placement[(e, q)]
            acc += _unpack(core, slot, m)
        out[tok_idx[e][:m]] += acc * tok_w[e][:m, None]
        if cnt > m:
            # overflow part lives in unit E on one core-half
            k = cnt - m
            half = 0 if e == ranks[0] else 1
            acc = np.zeros((k, C), dtype=np.float32)
            for q in range(4):
                acc += _unpack(4 * half + q, "E", k)
            out[tok_idx[e][m:]] += acc * tok_w[e][m:, None]
    return out.reshape(Bq, Nq, C).astype(x.dtype)



# revision 53
# speedup vs baseline: 1.0031x; 1.0031x over previous
"""MoE layer (top-k routing) on 8 Trainium2 NeuronCores.

Expert-parallel per the sharding hint: the host computes router softmax +
top-k (0.1% of FLOPs) and realizes the "all-to-all dispatch by expert
assignment" while building the per-core SPMD input maps; each core runs
expert FFN work in bf16 (fp32 PSUM accumulation); the host applies the
combine weights and scatter-adds results back to [B,N,C].

Load balance: each expert's FFN is split along D_FF into four quarter-units
(exact: gelu is elementwise over F and GEMM2 contracts F, so the four
partial y's just add). The 32 quarter-units are assigned four per core, one
per slot class A-D: slot A holds the two largest experts' quarters, slot B
the next two, etc. Each slot is padded to the max count within its pair, so
per-core padded work is sum over slots of max(pair) — within ~1% of the
perfect-balance floor — instead of 2*max(all counts).

Perf notes (measured on HW, ~922.5us vs ~874us pure-matmul floor):
- PE is the bottleneck: 95%+ matmul occupancy, 215ns per 128x128x512
  bf16 matmul (512 cycles @2.4GHz + 2.5ns NX dispatch floor). fp8 would
  halve compute but fails the 2e-2 rel-err gate (measured 4.6e-2);
  1024-wide moving ops fail walrus ISA codegen (s3d3_mm_num_elements).
- The startup window (~9-30us) is DMA-limited while the DMA subsystem
  ramps (~125GB/s aggregate early): tile-0's x0 + w1A + w2A (5MB) are
  chunked per contract-block across all three issuing queues (sync /
  scalar / gpsimd) in consumption order. The scalar queue carries only
  4 early dma_starts - each one blocks the ACT engine ~2us and would
  starve gelu otherwise.
- All DRAM tensors are host-repacked: x/y chunk-major [128,nt,8,512]
  (8KB descriptors/partition-line per tile), weights [128,8,1024] (16KB),
  biases pre-transposed into one [128,64] blob (one 128-descriptor DMA
  vs 8x1024 4-byte scatters). DMA engines are descriptor-rate-bound, so
  fat descriptors matter more than queue choice.
- 20 zero-input warm-up matmuls run during the load window: a cold/idled
  PE runs 2-3x slow (427-609ns) for its first ~15 matmuls.
- x tiles 0-2 prefetch ahead of the 12MB B-D weight bulk (an 8.8us PE
  stall when queued behind it); the bulk is split sync/gpsimd.
- Remaining gap to floor: ~9us fixed NEFF preamble + ring start, ~7us
  of early DMA-ramp stalls, ~10.9us NX dispatch tax, ~7us pair-padding
  (SPMD uniform caps), ~5us drain. All four variants of early-window
  scheduling measured within +/-1.5us - it is delivery-bound, not
  schedule-bound.
"""

import json
import os
import sys
import types

import numpy as np
import ml_dtypes

D_MODEL = 1024
D_FF = 4096
N_EXPERTS = 8
N_CORES = 8

P = 128
CB = D_MODEL // P      # 8 c-blocks of 128
FQ = D_FF // 4         # F quarter = 1024
FBQ = FQ // P          # 8 f-blocks per quarter
TN = 512               # token tile (matmul moving free dim / one PSUM bank)
SLOTS = ("A", "B", "C", "D", "E")   # E = overflow unit (tiny)


def _shim_axon_hooks():
    """Register the NTFF profile hook bass_utils looks for under axon; the
    image's `antenv` stub lacks `axon_hooks`."""
    if "antenv.axon_hooks" in sys.modules:
        return
    try:
        import trn_agent_boot.trn_boot as _tb
        hook = _tb._ntff_profile_via_ctypes("/opt/axon/libaxon_pjrt.so")
    except Exception:
        hook = None
    mod = types.ModuleType("antenv.axon_hooks")
    mod.get_axon_ntff_profile_hook = lambda: hook
    mod.set_axon_ntff_profile_hook = lambda h: None
    sys.modules["antenv.axon_hooks"] = mod


_shim_axon_hooks()

import concourse.bass as bass            # noqa: E402
import concourse.tile as tile            # noqa: E402
from concourse import mybir              # noqa: E402
from concourse.bass import ds, ts        # noqa: E402
from concourse.bass_utils import run_bass_kernel_spmd  # noqa: E402


def _fix_multiwait_bir(nc):
    """Split instructions carrying >1 sync wait (the TileContext tail drain)
    into single-wait NoOps; this walrus build rejects multi-wait CTRL
    instructions."""
    raw = bass.Bass.to_json_bytes(nc)
    d = json.loads(raw)
    for f in d["functions"]:
        for b in f["blocks"]:
            out = []
            for i in b["instructions"]:
                si = i.get("sync_info") or {}
                waits = si.get("on_wait") or []
                if len(waits) > 1:
                    for k, w in enumerate(waits[:-1]):
                        out.append({
                            "name": f"{i['name']}_wsplit{k}",
                            "engine": i["engine"],
                            "ins": [], "outs": [],
                            "opcode": "NoOp",
                            "sync_info": {"on_update": [], "on_wait": [w]},
                        })
                    si["on_wait"] = [waits[-1]]
                out.append(i)
            b["instructions"] = out
    fixed = json.dumps(d).encode()
    nc.to_json_bytes = lambda: fixed


_NC_CACHE = {}


def _token_tiles(cap, small_first=False):
    # small_first: a 256-token leading tile halves the bytes the very first
    # matmul waits on; later tiles are prefetched early enough to cover it
    tiles, off = [], 0
    if small_first and cap > TN:
        tiles.append((0, 256))
        off = 256
    while off < cap:
        tw = min(TN, cap - off)
        tiles.append((off, tw))
        off += tw
    return tiles


def _build_moe_kernel(caps):
    """Four quarter-expert FFN units per core (slots A-D), SPMD x8."""
    key = tuple(caps)
    if key in _NC_CACHE:
        return _NC_CACHE[key]

    bf16 = mybir.dt.bfloat16
    f32 = mybir.dt.float32
    Act = mybir.ActivationFunctionType

    nc = bass.Bass("TRN2", target_bir_lowering=False, debug=False,
                   num_devices=N_CORES)

    # all 8 per-slot bias vectors pre-rearranged by the host into one
    # [128, 64] blob: one 128-descriptor DMA instead of 8 DMAs of 1024
    # 4-byte descriptors each (~0.8us queue time + heavy DMA-engine load)
    biasP = nc.declare_dram_parameter("bias", [P, len(SLOTS) * (FBQ + CB)],
                                      f32, isOutput=False)
    # All DRAM tensors are host-repacked to partition-major layouts so each
    # DMA descriptor covers a full per-partition line (8-16KB) instead of a
    # 1-2KB row segment: the DMA system is descriptor-rate-bound (~80ns per
    # descriptor per engine), not byte-bound, so fat descriptors are what
    # make loads land fast.
    units = []
    for si, (slot, cap) in enumerate(zip(SLOTS, caps)):
        u = {"cap": cap, "slot": slot, "boff": si * (FBQ + CB)}
        u["tiles"] = _token_tiles(cap)
        nt = len(u["tiles"])
        u["nt"] = nt
        # tiny units (overflow slot E) get a 128-wide DRAM chunk so the
        # final store moves 0.25MB, not a 1MB zero-padded chunk
        cw = TN if cap > P else P
        u["cw"] = cw
        # x/y chunk-major: [128, ntiles, 8, cw] -> per-(p,tile) fat runs
        u["xT"] = nc.declare_dram_parameter(f"xT{slot}", [P, nt, CB, cw], bf16, isOutput=False)
        # w: [128, 8, 1024] -> per-p 16KB runs
        u["w1t"] = nc.declare_dram_parameter(f"w1t{slot}", [P, CB, FQ], bf16, isOutput=False)
        u["w2t"] = nc.declare_dram_parameter(f"w2t{slot}", [P, FBQ, D_MODEL], bf16, isOutput=False)
        # partials return as bf16: halves the output DMA so total traffic
        # stays under the chip's P0 power-throttle trigger (observed: the
        # f32 version pushed PE from 2.4 to 2.0 GHz); host sums in f32
        u["yT"] = nc.declare_dram_parameter(f"yT{slot}", [P, nt, CB, cw], bf16, isOutput=True)
        u["xr"] = u["xT"].ap()      # [128, nt, 8, 512]
        u["w1r"] = u["w1t"].ap()    # [128, 8, 1024]
        u["w2r"] = u["w2t"].ap()    # [128, 8, 1024]
        u["yr"] = u["yT"].ap()
        units.append(u)

    with tile.TileContext(nc) as tc:
        with (
            tc.tile_pool(name="weights", bufs=1) as wpool,
            tc.tile_pool(name="xin", bufs=3) as xpool,
            tc.tile_pool(name="hbuf", bufs=1) as hpool,
            tc.tile_pool(name="yout", bufs=3) as ypool,
            tc.tile_pool(name="psum", bufs=4, space="PSUM") as psum,
            tc.tile_pool(name="warm", bufs=1) as warmpool,
        ):
            # PE clock warm-up: ~20 dependency-free matmuls on a memset tile
            # run while the first loads are in flight. A cold PE runs its
            # first ~15 matmuls at half clock (427-609ns vs 215ns); burning
            # the ramp on dummies makes the real tile-0 matmuls full-speed.
            wsrc = warmpool.tile([P, P + TN], bf16, tag="wsrc", name="wsrc")
            nc.vector.memset(wsrc[:, :], 0)
            # borrow a rotation slot from the main psum pool's ph tag; the
            # warm matmuls retire long before the 4-deep rotation reuses it
            wps = psum.tile([P, TN], f32, tag="ph", name="wps")
            for _ in range(20):
                nc.tensor.matmul(wps[:, :], lhsT=wsrc[:, 0:P],
                                 rhs=wsrc[:, P:P + TN], start=True, stop=True)
            # ---- loads. SP-ring FIFO order is chosen so PE never waits:
            # tile-0 tokens + first w1 strip first (w1 strip on the ACT ring
            # so it overlaps x0's load), then unit A's remaining weights,
            # then unit A's tile-1 tokens BEFORE units B-D's weight bulk so
            # early tiles stay ahead of the PE.
            # chunked so the first matmul waits on 128KB, not a 1MB monolith:
            # x0 per-k on the sync ring; w1A/w2A chunked along the contract
            # group g (keeps 2KB descriptors — chunking along f/c would 8x
            # the descriptor count). w1A is split across the gpsimd and
            # scalar rings so G1(tile0) is fed at ~2 rings' bandwidth. The
            # scalar queue gets only 4 early dma_starts (issued well before
            # the first gelu needs the engine).
            ua = units[0]
            ua["x0"] = xpool.tile([P, CB, TN], bf16, tag="xt", name="x0A")
            ua["w1_sb"] = wpool.tile([P, CB, FQ], bf16, tag="w1A", name="w1A")
            # early window is aggregate-DMA-ramp-limited (~125GB/s): feed
            # tile-0's consumption order with many small per-g chunks spread
            # across all three rings (empirically the fastest variant): x0
            # per-g on sync, w1A per-g alternating gpsimd/scalar, w2A per-g
            # on gpsimd behind the bias blob.
            bias_sb = wpool.tile([P, len(SLOTS) * (FBQ + CB)], f32, tag="bias",
                                 name="bias")
            nc.gpsimd.dma_start(bias_sb[:, :], biasP.ap())
            for g in range(CB):
                # first-needed chunks on scalar: its ring starts ~2us
                # earlier than gpsimd's
                ring = nc.scalar if g < 4 else nc.gpsimd
                ring.dma_start(ua["w1_sb"][:, g, :], ua["w1r"][:, g, :])
            for g in range(CB):
                nc.sync.dma_start(ua["x0"][:, g, :], ua["xr"][:, 0, g, :])
            ua["w2_sb"] = wpool.tile([P, FBQ, D_MODEL], bf16, tag="w2A", name="w2A")
            for g in range(FBQ):
                nc.gpsimd.dma_start(ua["w2_sb"][:, g, :], ua["w2r"][:, g, :])

            # unit A tiles 1 and 2 ahead of the remaining weight bulk (the
            # tile-2 load sat behind 12MB of B-D weights on the sync ring and
            # cost an 8.8us PE stall)
            for ti in (1, 2):
                if len(ua["tiles"]) > ti:
                    xt = xpool.tile([P, CB, TN], bf16, tag="xt", name=f"x{ti}A")
                    nc.sync.dma_start(xt[:, :, :], ua["xr"][:, ti, :, :])
                    ua[f"x{ti}"] = xt

            # B/C weight bulk split across the sync and gpsimd rings. Units
            # D and E REUSE earlier units' weight buffers via tag sharing
            # (saves 64KB/partition of SBUF): their loads sit on the
            # otherwise-idle gpsimd queue and block there until the donor's
            # last read — donors free at ~217-690us, consumers start at
            # ~690-910us, so the serialization is harmless.
            for u in units[1:3]:
                slot = u["slot"]
                u["w1_sb"] = wpool.tile([P, CB, FQ], bf16, tag=f"w1{slot}",
                                        name=f"w1{slot}")
                u["w2_sb"] = wpool.tile([P, FBQ, D_MODEL], bf16, tag=f"w2{slot}",
                                        name=f"w2{slot}")
            ud, ue = units[3], units[4]
            for u in units[1:3]:
                nc.sync.dma_start(u["w1_sb"][:, :, :], u["w1r"][:, :, :])
                nc.gpsimd.dma_start(u["w2_sb"][:, :, :], u["w2r"][:, :, :])

            # ---- compute: unit A..E tiles in sequence. Deferred weight
            # loads for D and E reuse donor buffers (tag sharing) and MUST
            # be emitted after the donor unit's matmuls so the WAR hazard
            # is tracked in program order; they sit on the idle gpsimd
            # queue and wait there for the donor's last read.
            for ui, u in enumerate(units):
                if ui == 3:
                    # donors: w1A free after A's G1, w2B free after B's G2
                    ud["w1_sb"] = wpool.tile([P, CB, FQ], bf16, tag="w1A",
                                             name="w1D")
                    nc.gpsimd.dma_start(ud["w1_sb"][:, :, :], ud["w1r"][:, :, :])
                    ud["w2_sb"] = wpool.tile([P, FBQ, D_MODEL], bf16,
                                             tag="w2B", name="w2D")
                    nc.gpsimd.dma_start(ud["w2_sb"][:, :, :], ud["w2r"][:, :, :])
                if ui == 4:
                    ue["w1_sb"] = wpool.tile([P, CB, FQ], bf16, tag="w1B",
                                             name="w1E")
                    nc.gpsimd.dma_start(ue["w1_sb"][:, :, :], ue["w1r"][:, :, :])
                    ue["w2_sb"] = wpool.tile([P, FBQ, D_MODEL], bf16,
                                             tag="w2C", name="w2E")
                    nc.gpsimd.dma_start(ue["w2_sb"][:, :, :], ue["w2r"][:, :, :])
                for ti, (off, tw) in enumerate(u["tiles"]):
                    if f"x{ti}" in u:
                        xt = u[f"x{ti}"]
                    else:
                        xt = xpool.tile([P, CB, TN], bf16, tag="xt")
                        nc.sync.dma_start(xt[:, :, :u["cw"]],
                                          u["xr"][:, ti, :, :])

                    ht = hpool.tile([P, FBQ, TN], bf16, tag="ht")
                    for m in range(FBQ):
                        ph = psum.tile([P, TN], f32, tag="ph")
                        for k in range(CB):
                            nc.tensor.matmul(
                                ph[:, :tw],
                                lhsT=u["w1_sb"][:, k, ts(m, P)],
                                rhs=xt[:, k, :tw],
                                start=(k == 0), stop=(k == CB - 1),
                            )
                        nc.scalar.activation(ht[:, m, :tw], ph[:, :tw], Act.Gelu,
                                             bias=bias_sb[:, u["boff"] + m:u["boff"] + m + 1])

                    last = (u is units[-1]) and (ti == len(u["tiles"]) - 1)
                    yt = ypool.tile([P, CB, TN], bf16, tag="yt")
                    for c in range(CB):
                        py = psum.tile([P, TN], f32, tag="py")
                        for k in range(FBQ):
                            nc.tensor.matmul(
                                py[:, :tw],
                                lhsT=u["w2_sb"][:, k, ts(c, P)],
                                rhs=ht[:, k, :tw],
                                start=(k == 0), stop=(k == FBQ - 1),
                            )
                        bo = u["boff"] + FBQ
                        nc.scalar.add(yt[:, c, :tw], py[:, :tw],
                                      bias_sb[:, bo + c:bo + c + 1])
                        if last and tw >= 256:
                            # final tile: per-block stores overlap the tail
                            # GEMM2 blocks instead of one post-loop DMA
                            nc.sync.dma_start(u["yr"][:, ti, c, :tw],
                                              yt[:, c, :tw])
                    if not last or tw < 256:
                        # full-chunk store (8KB descriptors); the pad zone
                        # carries stale data the host never reads. For a
                        # small final tile one store beats 8 serial 0.6us
                        # DIRECT2D issues after the last matmul.
                        nc.sync.dma_start(u["yr"][:, ti, :, :],
                                          yt[:, :, :u["cw"]])

    _fix_multiwait_bir(nc)
    _NC_CACHE[key] = nc
    return nc


def _route(xf, router_w, k):
    """Replicate the reference router numerics (f32 softmax, top-k, renorm)."""
    logits = xf @ router_w.T.astype(np.float32)          # [T, E]
    m = logits.max(axis=-1, keepdims=True)
    e = np.exp(logits - m, dtype=np.float32)
    probs = e / e.sum(axis=-1, keepdims=True)
    # descending, ties -> lower index first (matches jax.lax.top_k)
    idx = np.argsort(-probs, axis=-1, kind="stable")[:, :k]   # [T, k]
    w = np.take_along_axis(probs, idx, axis=-1)               # [T, k]
    w = w / (w.sum(axis=-1, keepdims=True) + 1e-9)
    return idx, w


def _align16(n):
    # 4-token (8-byte) alignment keeps DMA rows aligned; finer than 16
    # saves ~20 padded tokens across the four slots
    return max(P, -(-n // 4) * 4)


def kernel(x, router_w, expert_w1, expert_b1, expert_w2, expert_b2, top_k):
    x = np.asarray(x)
    router_w = np.asarray(router_w, dtype=np.float32)
    expert_w1 = np.asarray(expert_w1, dtype=np.float32)
    expert_b1 = np.asarray(expert_b1, dtype=np.float32)
    expert_w2 = np.asarray(expert_w2, dtype=np.float32)
    expert_b2 = np.asarray(expert_b2, dtype=np.float32)
    k = int(np.asarray(top_k))
    Bq, Nq, C = x.shape
    Tq = Bq * Nq
    E = expert_w1.shape[0]
    xf = np.ascontiguousarray(x.reshape(Tq, C), dtype=np.float32)

    idx, w = _route(xf, router_w, k)

    tok_idx, tok_w = [], []
    for e in range(E):
        mask = idx == e
        sel = np.nonzero(mask.any(axis=-1))[0]
        tok_idx.append(sel)
        tok_w.append((w * mask).sum(axis=-1)[sel].astype(np.float32))
    counts = np.array([len(s) for s in tok_idx])

    # Slots A-D pair experts by rank but pad only to the SMALLER member of
    # the A and D pairs; the two truncated experts' overflow tokens (rank 0
    # on cores 0-3, rank 6 on cores 4-7 — slot D is swapped so rank 6 sits
    # on the 4-7 half) run as a tiny fifth unit E with its own weights.
    order = np.argsort(-counts, kind="stable")
    ranks = [int(order[i]) for i in range(8)]
    caps = [_align16(int(counts[ranks[1]])),
            _align16(int(counts[ranks[2]])),
            _align16(int(counts[ranks[4]])),
            _align16(int(counts[ranks[7]]))]
    ov_a = int(counts[ranks[0]]) - caps[0]
    ov_d = int(counts[ranks[6]]) - caps[3]
    caps.append(max(4, -(-max(ov_a, ov_d, 1) // 4) * 4))

    nc = _build_moe_kernel(tuple(caps))

    def _pack(tmp, nt, w=TN):
        return np.ascontiguousarray(
            tmp.reshape(nt, w, CB, P).transpose(3, 0, 2, 1)
        ).astype(ml_dtypes.bfloat16)

    # slot -> (expert on cores 0-3, expert on cores 4-7)
    slot_experts = [(ranks[0], ranks[1]), (ranks[2], ranks[3]),
                    (ranks[4], ranks[5]), (ranks[7], ranks[6])]

    # one xT per expert (first main_cap tokens), shared by its 4 quarters
    xTs, main_of = {}, {}
    for s in range(4):
        nt = (caps[s] + TN - 1) // TN
        for e in slot_experts[s]:
            m = min(int(counts[e]), caps[s])
            main_of[e] = (s, m)
            tmp = np.zeros((nt * TN, C), dtype=np.float32)
            tmp[:m] = xf[tok_idx[e][:m]]
            xTs[e] = _pack(tmp, nt)
    # overflow xT per core-half (128-wide chunk when cap_E <= 128)
    nt_e = (caps[4] + TN - 1) // TN
    w_e = TN if caps[4] > P else P
    xTov = []
    for e in (ranks[0], ranks[6]):
        s, m = main_of[e]
        k = int(counts[e]) - m
        tmp = np.zeros((nt_e * w_e, C), dtype=np.float32)
        if k > 0:
            tmp[:k] = xf[tok_idx[e][m:]]
        xTov.append(_pack(tmp, nt_e, w_e))

    in_maps = [dict() for _ in range(N_CORES)]
    placement = {}          # (expert, quarter) -> (core, slot name)
    FBQ_, CB_ = FQ // P, C // P
    blobs = [np.zeros((P, len(SLOTS) * (FBQ_ + CB_)), dtype=np.float32)
             for _ in range(N_CORES)]
    for s, slot in enumerate(SLOTS):
        for core in range(N_CORES):
            q = core % 4
            half = 0 if core < 4 else 1
            if s < 4:
                e = slot_experts[s][half]
                placement[(e, q)] = (core, slot)
                xarr = xTs[e]
            else:
                e = ranks[0] if half == 0 else ranks[6]
                xarr = xTov[half]
            f0, f1 = q * FQ, (q + 1) * FQ
            b2 = expert_b2[e] if q == 0 else np.zeros(C, dtype=np.float32)
            bo = s * (FBQ_ + CB_)
            blobs[core][:, bo:bo + FBQ_] = expert_b1[e, f0:f1].reshape(FBQ_, P).T
            blobs[core][:, bo + FBQ_:bo + FBQ_ + CB_] = b2.reshape(CB_, P).T
            w1q = expert_w1[e, f0:f1].T.reshape(CB_, P, FQ).transpose(1, 0, 2)
            w2q = expert_w2[e, :, f0:f1].T.reshape(FBQ_, P, C).transpose(1, 0, 2)
            in_maps[core].update({
                f"xT{slot}": xarr,
                f"w1t{slot}": np.ascontiguousarray(w1q).astype(ml_dtypes.bfloat16),
                f"w2t{slot}": np.ascontiguousarray(w2q).astype(ml_dtypes.bfloat16),
            })
    for core in range(N_CORES):
        in_maps[core]["bias"] = blobs[core]

    trace = os.environ.get("BASS_MOE_TRACE") == "1"
    res = run_bass_kernel_spmd(
        nc, in_maps, core_ids=list(range(N_CORES)),
        trace=trace,
        tmpdir=os.environ.get("BASS_MOE_TMPDIR") if trace else None,
    )
    if trace:
        kernel.last_exec_time_ns = res.exec_time_ns
        kernel.last_trace = (res.instructions_and_trace or (None, None))[1]

    def _unpack(core, slot, take):
        yq = res.results[core][f"yT{slot}"]            # [128, nt, 8, cw]
        return yq.transpose(1, 3, 2, 0).reshape(-1, C)[:take].astype(np.float32)

    out = np.zeros((Tq, C), dtype=np.float32)
    for e in range(E):
        cnt = int(counts[e])
        if not cnt:
            continue
        _, m = main_of[e]
        acc = np.zeros((m, C), dtype=np.float32)
        for q in range(4):
            core, slot = placement[(e, q)]
            acc += _unpack(core, slot, m)
        out[tok_idx[e][:m]] += acc * tok_w[e][:m, None]
        if cnt > m:
            # overflow part lives in unit E on one core-half
            k = cnt - m
            half = 0 if e == ranks[0] else 1
            acc = np.zeros((k, C), dtype=np.float32)
            for q in range(4):
                acc += _unpack(4 * half + q, "E", k)
            out[tok_idx[e][m:]] += acc * tok_w[e][m:, None]
    return out.reshape(Bq, Nq, C).astype(x.dtype)

